# revision 14
# baseline (speedup 1.0000x reference)
"""Graph-transformer block on 8 Trainium2 NeuronCores.

Sharding: each core takes a 512-row q-slice of the 4096 nodes across ALL 4
heads. No cross-core communication.

v3 design:
- adj host-transposed per core into SBUF layout
  [128p(j%128), hd*16384 + jb*512 + q] bf16; 16 x 1MB contiguous DMAs.
- S^T [j, q] tiles computed directly on the PE (no transposes anywhere).
- P = exp(s*scale*adj) = 1 + adj*(exp(s*scale)-1):
    ACT: e = exp(st * SCALE) from PSUM, 1024-wide (amortize +352/instr)
    DVE: em1 = e - 1 (tensor_scalar, 4x mode, 2048-wide)
         pt  = em1 * adjT (tensor_tensor, 2x mode, 2048-wide)
    PE:  X'[hd] += V'[jb].T @ pt-slices (PSUM accumulation)
  V' has a ones-column per head block so X' row 64 = denom - N.
  X' is init'd with Vsum via matmul of Wv' against broadcast hsum.
- Finalize per head uses 1/(N+d) ~= 1/N - d/N^2 (|d|<~20, rel err <2e-5):
  one 1-lane tensor_scalar + gpsimd partition_broadcast + one tensor_tensor.
- FFN: relu done on DVE (tensor_scalar add-bias + max0); row softmax skips
  max-subtraction (|logits| < 0.01).
"""
import sys
import numpy as np

sys.path.insert(0, "/opt/trn_rl_repo")
import ml_dtypes  # noqa: E402

IN = 256
H = 4
DH = 64
NCORES = 8
F1 = 512
DOUT = 256
SCALE = 1.0 / 16.0  # 1/sqrt(IN)
BF16 = ml_dtypes.bfloat16

_cache = {}


def build(n_nodes=4096, qs=512):
    key = (n_nodes, qs)
    if key in _cache:
        return _cache[key]

    from contextlib import ExitStack
    import concourse.tile as tile
    from concourse import mybir, bacc
    from concourse.alu_op_type import AluOpType

    fp32, bf16 = mybir.dt.float32, mybir.dt.bfloat16
    AF = mybir.ActivationFunctionType
    AX = mybir.AxisListType

    NJB = n_nodes // 128          # 128-row j blocks (32)
    NJC = NJB // 8                # 1MB adj chunks per head (4)
    NQC = qs // 128               # 128-row q chunks (4)
    VW = 65                       # v block width per head (64 + ones col)
    RN = 1.0 / float(n_nodes)     # 1/4096
    RN2 = RN * RN

    nc = bacc.Bacc("TRN2", target_bir_lowering=False, debug=False,
                   enable_asserts=False)

    adj_d = nc.dram_tensor("adjc", [128, H * NJB * qs], bf16,
                           kind="ExternalInput").ap()
    hT_d = nc.dram_tensor("hT", [IN, n_nodes], bf16, kind="ExternalInput").ap()
    hTq_d = nc.dram_tensor("hTq", [IN, qs], bf16, kind="ExternalInput").ap()
    hs_d = nc.dram_tensor("hs", [IN, 1], bf16, kind="ExternalInput").ap()
    wq_d = nc.dram_tensor("wq", [IN, H * DH], bf16, kind="ExternalInput").ap()
    wk_d = nc.dram_tensor("wk", [IN, H * DH], bf16, kind="ExternalInput").ap()
    wv_d = nc.dram_tensor("wv", [IN, H * VW], bf16, kind="ExternalInput").ap()
    w1_d = nc.dram_tensor("w1", [IN, F1], bf16, kind="ExternalInput").ap()
    w2_d = nc.dram_tensor("w2", [F1, DOUT], bf16, kind="ExternalInput").ap()
    b1_d = nc.dram_tensor("b1", [128, F1 // 128], fp32, kind="ExternalInput").ap()
    b2_d = nc.dram_tensor("b2", [1, DOUT], fp32, kind="ExternalInput").ap()
    out_d = nc.dram_tensor("out", [qs, DOUT], fp32, kind="ExternalOutput").ap()

    with ExitStack() as ctx:
        tc = ctx.enter_context(tile.TileContext(nc))
        pc = ctx.enter_context(tc.tile_pool(name="const", bufs=1))
        pst = ctx.enter_context(tc.tile_pool(name="stp", bufs=3, space="PSUM"))
        pxt = ctx.enter_context(tc.tile_pool(name="xtp", bufs=2, space="PSUM"))
        pa = ctx.enter_context(tc.tile_pool(name="adjp", bufs=4))
        pe_ = ctx.enter_context(tc.tile_pool(name="ep", bufs=3))
        pm1 = ctx.enter_context(tc.tile_pool(name="m1p", bufs=2))
        ppt = ctx.enter_context(tc.tile_pool(name="ptp", bufs=2))
        psm = ctx.enter_context(tc.tile_pool(name="smallp", bufs=2))

        # ---------------- input DMAs ----------------
        # order on the sync HWDGE ring = completion order: small critical
        # prep inputs first (qT needs hTq+wq), then hT (kT/v), then the rest;
        # adj chunks queue behind all of these.
        hTq_sb = [pc.tile([128, qs], bf16, tag=f"hTq{dc}", name=f"hTq{dc}")
                  for dc in range(2)]
        for dc in range(2):
            nc.sync.dma_start(out=hTq_sb[dc][:], in_=hTq_d[dc * 128:(dc + 1) * 128, :])
        wq_sb = pc.tile([128, 2 * H * DH], bf16, tag="wq")
        wk_sb = pc.tile([128, 2 * H * DH], bf16, tag="wk")
        for sb, d in ((wq_sb, wq_d), (wk_sb, wk_d)):
            for dc in range(2):
                nc.sync.dma_start(out=sb[:, dc * 256:(dc + 1) * 256],
                                    in_=d[dc * 128:(dc + 1) * 128, :])
        hT_sb = [pc.tile([128, n_nodes], bf16, tag=f"hT{dc}", name=f"hT{dc}")
                 for dc in range(2)]
        for dc in range(2):
            nc.sync.dma_start(out=hT_sb[dc][:], in_=hT_d[dc * 128:(dc + 1) * 128, :])
        wv_sb = pc.tile([128, 2 * H * VW], bf16, tag="wv")
        for dc in range(2):
            nc.sync.dma_start(out=wv_sb[:, dc * 260:(dc + 1) * 260],
                                in_=wv_d[dc * 128:(dc + 1) * 128, :])
        w1_sb = [pc.tile([128, F1], bf16, tag=f"w1_{dc}", name=f"w1_{dc}")
                 for dc in range(2)]
        for dc in range(2):
            nc.sync.dma_start(out=w1_sb[dc][:], in_=w1_d[dc * 128:(dc + 1) * 128, :])
        w2_sb = pc.tile([128, 4 * DOUT], bf16, tag="w2")
        for fc in range(4):
            nc.sync.dma_start(out=w2_sb[:, fc * DOUT:(fc + 1) * DOUT],
                                in_=w2_d[fc * 128:(fc + 1) * 128, :])
        b1_sb = pc.tile([128, F1 // 128], fp32, tag="b1")
        nc.sync.dma_start(out=b1_sb[:], in_=b1_d[:, :])
        b2_sb = pc.tile([1, DOUT], fp32, tag="b2")
        nc.sync.dma_start(out=b2_sb[:], in_=b2_d[:, :])
        b2_bc = pc.tile([128, DOUT], fp32, tag="b2_bc")
        nc.gpsimd.partition_broadcast(b2_bc[:], b2_sb[0:1, :])

        # ---------------- projections ----------------
        # q^T / k^T: head pairs packed on partitions (pair p -> heads 2p,2p+1)
        qT_sb = [pc.tile([128, qs], bf16, tag=f"qT{p}", name=f"qT{p}") for p in range(2)]
        qps = pst.tile([128, 1024], fp32, tag="st")
        for p in range(2):
            for dc in range(2):
                nc.tensor.matmul(qps[:, p * 512:(p + 1) * 512],
                                 wq_sb[:, dc * 256 + p * 128: dc * 256 + (p + 1) * 128],
                                 hTq_sb[dc][:],
                                 start=(dc == 0), stop=(dc == 1))
        for p in range(2):
            nc.scalar.activation(qT_sb[p][:], qps[:, p * 512:(p + 1) * 512], AF.Copy)
        # kT: wide rhs (N=1024), copies on ACT (idle during prep)
        kT_sb = [pc.tile([128, n_nodes], bf16, tag=f"kT{p}", name=f"kT{p}") for p in range(2)]
        for p in range(2):
            for jt in range(n_nodes // 1024):
                ps = pst.tile([128, 1024], fp32, tag="st")
                for s in range(2):
                    for dc in range(2):
                        nc.tensor.matmul(ps[:, s * 512:(s + 1) * 512],
                                         wk_sb[:, dc * 256 + p * 128: dc * 256 + (p + 1) * 128],
                                         hT_sb[dc][:, jt * 1024 + s * 512: jt * 1024 + (s + 1) * 512],
                                         start=(dc == 0), stop=(dc == 1))
                nc.scalar.activation(kT_sb[p][:, jt * 1024:(jt + 1) * 1024], ps[:], AF.Copy)
        # v natural [128j, NJB*260] bf16; block jb cols jb*260 + hd*65 + f,
        # col jb*260 + hd*65 + 64 = 1.0 (ones col; wv has zeros there)
        v_sb = pc.tile([128, NJB * H * VW], bf16, tag="v")
        for jb2 in range(NJB // 2):
            ps = pst.tile([128, 520], fp32, tag="st")
            for s in range(2):
                jb = jb2 * 2 + s
                for dc in range(2):
                    nc.tensor.matmul(ps[:, s * 260:(s + 1) * 260],
                                     hT_sb[dc][:, jb * 128:(jb + 1) * 128],
                                     wv_sb[:, dc * 260:(dc + 1) * 260],
                                     start=(dc == 0), stop=(dc == 1))
            nc.vector.tensor_copy(v_sb[:, jb2 * 520:(jb2 + 1) * 520], ps[:])
        # set every ones-col (all cols == 64 mod 65) to 1.0 in one strided memset
        nc.gpsimd.memset(v_sb[:, 64::65], 1.0)

        # ---------------- Vsum = Wv'.T @ hsum (hsum shipped from host) ----
        hsum_bf = [psm.tile([128, 1], bf16, tag=f"hsumb{dc}", name=f"hsumb{dc}")
                   for dc in range(2)]
        for dc in range(2):
            nc.sync.dma_start(out=hsum_bf[dc][:], in_=hs_d[dc * 128:(dc + 1) * 128, :])
        # Vsum[hd] = sum_j v'[j, hd, :] as [65, 1] per-partition columns
        vs_ps = pst.tile([VW, H], fp32, tag="st")
        for hd in range(H):
            for dc in range(2):
                nc.tensor.matmul(vs_ps[:, hd:hd + 1],
                                 wv_sb[:, dc * 260 + hd * VW: dc * 260 + hd * VW + VW],
                                 hsum_bf[dc][:],
                                 start=(dc == 0), stop=(dc == 1))
        vs_sb = psm.tile([VW, H], fp32, tag="vs_sb")
        nc.vector.tensor_copy(vs_sb[:], vs_ps[:])


        # ---------------- attention ----------------
        embT = [pc.tile([128, qs], bf16, tag=f"embT{p}", name=f"embT{p}") for p in range(2)]

        for pr in range(2):
            xts = [pxt.tile([VW, qs], fp32, tag="xt", name=f"xt{2 * pr + i}")
                   for i in range(2)]
            for jc in range(NJC):
                acs = []
                for i in range(2):
                    ac = pa.tile([128, 8 * 512], bf16, tag="adj")
                    base = ((2 * pr + i) * NJC + jc) * 8 * 512
                    nc.sync.dma_start(out=ac[:], in_=adj_d[:, base: base + 8 * 512])
                    acs.append(ac)
                es = [pe_.tile([128, 4096], bf16, tag="e",
                               name=f"e{pr}_{jc}_{i}") for i in range(2)]
                # 16 row-tiled S^T matmuls (heads 2pr/2pr+1 run on row
                # tiles (0,0)/(64,0) of the PE concurrently)
                for sub in range(4):
                    for i in range(2):
                        off = i * 64
                        st = pst.tile([128, 1024], fp32, tag="st")
                        for k in range(2):
                            jb = jc * 8 + sub * 2 + k
                            nc.tensor.matmul(st[:, k * 512:(k + 1) * 512],
                                             kT_sb[pr][off:off + 64, jb * 128:(jb + 1) * 128],
                                             qT_sb[pr][off:off + 64, :],
                                             start=True, stop=True,
                                             tile_position=(off, 0))
                        nc.scalar.activation(es[i][:, sub * 1024:(sub + 1) * 1024],
                                             st[:], AF.Exp, scale=SCALE)
                for i in range(2):
                    hd = 2 * pr + i
                    em1 = pm1.tile([128, 4096], bf16, tag="em1")
                    pt = ppt.tile([128, 4096], bf16, tag="pt")
                    for h2 in range(2):
                        lo, hi = h2 * 2048, (h2 + 1) * 2048
                        nc.vector.tensor_scalar_sub(em1[:, lo:hi], es[i][:, lo:hi], 1.0)
                        nc.vector.tensor_tensor(pt[:, lo:hi], em1[:, lo:hi],
                                                acs[i][:, lo:hi], AluOpType.mult)
                        for k in range(4):
                            jb = jc * 8 + h2 * 4 + k
                            kk = h2 * 4 + k
                            nc.tensor.matmul(xts[i][:],
                                             v_sb[:, jb * 260 + hd * VW: jb * 260 + hd * VW + VW],
                                             pt[:, kk * 512:(kk + 1) * 512],
                                             start=(jb == 0), stop=(jb == NJB - 1))
            for i in range(2):
                hd = 2 * pr + i
                off = i * 64
                # finalize: emb^T rows = (X'' + Vsum) * (1/N - d/N^2), d = X''[64]
                rec1 = psm.tile([1, qs], fp32, tag="rec1")
                nc.vector.tensor_scalar(rec1[:], xts[i][64:65, :], -RN2, RN,
                                        op0=AluOpType.mult, op1=AluOpType.add)
                recb = psm.tile([64, qs], fp32, tag="recb")
                nc.gpsimd.partition_broadcast(recb[:], rec1[0:1, :])
                nc.vector.scalar_tensor_tensor(embT[pr][off:off + 64, :], xts[i][0:64, :],
                                               vs_sb[0:64, hd:hd + 1], recb[:],
                                               AluOpType.add, AluOpType.mult)

        # ---------------- FFN + row softmax ----------------
        p1_sb = pc.tile([128, (F1 // 128) * qs], bf16, tag="p1")
        for fc2 in range(2):
            ps = pst.tile([128, 1024], fp32, tag="st")
            for s in range(2):
                fc = fc2 * 2 + s
                for dc in range(2):
                    nc.tensor.matmul(ps[:, s * 512:(s + 1) * 512],
                                     w1_sb[dc][:, fc * 128:(fc + 1) * 128],
                                     embT[dc][:], start=(dc == 0), stop=(dc == 1))
            for s in range(2):
                fc = fc2 * 2 + s
                nc.vector.tensor_scalar(p1_sb[:, fc * qs:(fc + 1) * qs],
                                        ps[:, s * 512:(s + 1) * 512],
                                        b1_sb[:, fc:fc + 1], 0.0,
                                        op0=AluOpType.add, op1=AluOpType.max)
        for qc in range(NQC):
            ps2 = pst.tile([128, DOUT], fp32, tag="st")
            for fc in range(F1 // 128):
                nc.tensor.matmul(ps2[:],
                                 p1_sb[:, fc * qs + qc * 128: fc * qs + (qc + 1) * 128],
                                 w2_sb[:, fc * DOUT:(fc + 1) * DOUT],
                                 start=(fc == 0), stop=(fc == F1 // 128 - 1))
            t2 = psm.tile([128, DOUT], fp32, tag="t2")
            nc.vector.tensor_tensor(t2[:], ps2[:], b2_bc[:], AluOpType.add)
            e2 = psm.tile([128, DOUT], fp32, tag="e2")
            sm = psm.tile([128, 1], fp32, tag="sm")
            nc.scalar.activation(e2[:], t2[:], AF.Exp, accum_out=sm[:])
            rc2 = psm.tile([128, 1], fp32, tag="rc2")
            nc.vector.reciprocal(rc2[:], sm[:])
            o = psm.tile([128, DOUT], fp32, tag="o")
            nc.vector.tensor_scalar_mul(o[:], e2[:], rc2[:])
            nc.sync.dma_start(out=out_d[qc * 128:(qc + 1) * 128, :], in_=o[:])

    nc.compile()
    _cache[key] = nc
    return nc


def make_in_maps(h, adj, Wq, Wk, Wv, W1, b1, W2, b2, n_nodes, qs, ncores):
    h = np.asarray(h, np.float32)
    adj = np.asarray(adj, np.float32)
    hT = np.ascontiguousarray(h.T.astype(BF16))
    hs = h.sum(0, dtype=np.float64).astype(np.float32).astype(BF16).reshape(IN, 1)
    WqP = np.ascontiguousarray(
        np.asarray(Wq, np.float32).transpose(1, 0, 2).reshape(IN, H * DH)).astype(BF16)
    WkP = np.ascontiguousarray(
        np.asarray(Wk, np.float32).transpose(1, 0, 2).reshape(IN, H * DH)).astype(BF16)
    WvT = np.asarray(Wv, np.float32).transpose(1, 0, 2)  # [IN, H, DH]
    WvP = np.zeros((IN, H * 65), dtype=BF16)
    for hd in range(H):
        WvP[:, hd * 65: hd * 65 + 64] = WvT[:, hd, :].astype(BF16)
    W1b = np.asarray(W1, np.float32).astype(BF16)
    W2b = np.asarray(W2, np.float32).astype(BF16)
    b1r = np.ascontiguousarray(np.asarray(b1, np.float32).reshape(F1 // 128, 128).T)
    b2r = np.asarray(b2, np.float32).reshape(1, DOUT)
    # adj -> per-core SBUF-ready layout [128, hd*NJB*qs + jb*qs + q] (bf16)
    au = adj.astype(BF16).view(np.uint16)  # [H, N, N]
    NJB = n_nodes // 128
    in_maps = []
    for c in range(ncores):
        q0 = c * qs
        A = au[:, q0:q0 + qs, :]                       # [H, qs, N] view
        R = A.reshape(H, qs, NJB, 128).transpose(3, 0, 2, 1)  # [128, H, NJB, qs]
        adjc = np.ascontiguousarray(R).reshape(128, H * NJB * qs).view(BF16)
        in_maps.append({
            "adjc": adjc,
            "hT": hT,
            "hTq": np.ascontiguousarray(hT[:, q0:q0 + qs]),
            "hs": hs,
            "wq": WqP, "wk": WkP, "wv": WvP,
            "w1": W1b, "w2": W2b, "b1": b1r, "b2": b2r,
        })
    return in_maps


def kernel(h, adj, Wq, Wk, Wv, W1, b1, W2, b2):
    import os
    n_nodes, qs = 4096, 512
    nc = build(n_nodes, qs)
    from concourse.bass_utils import run_bass_kernel_spmd
    in_maps = make_in_maps(h, adj, Wq, Wk, Wv, W1, b1, W2, b2, n_nodes, qs, NCORES)
    trace = bool(os.environ.get("BASS_KERNEL_TRACE"))
    res = run_bass_kernel_spmd(nc, in_maps, list(range(NCORES)), trace=trace)
    if trace and res.exec_time_ns is not None:
        print(f"HW exec time: {res.exec_time_ns} ns")
        kernel.last_exec_time_ns = res.exec_time_ns
    out = np.concatenate([np.asarray(res.results[c]["out"]) for c in range(NCORES)],
                         axis=0)
    return out.astype(np.float32)


# revision 15
# speedup vs baseline: 1.1902x; 1.1902x over previous
"""Graph-transformer block on 8 Trainium2 NeuronCores.

Sharding: each core takes a 512-row q-slice of the 4096 nodes across ALL 4
heads. No cross-core communication.

v3 design:
- adj host-transposed per core into SBUF layout
  [128p(j%128), hd*16384 + jb*512 + q] bf16; 16 x 1MB contiguous DMAs.
- S^T [j, q] tiles computed directly on the PE (no transposes anywhere).
- P = exp(s*scale*adj) = 1 + adj*(exp(s*scale)-1):
    ACT: e = exp(st * SCALE) from PSUM, 1024-wide (amortize +352/instr)
    DVE: em1 = e - 1 (tensor_scalar, 4x mode, 2048-wide)
         pt  = em1 * adjT (tensor_tensor, 2x mode, 2048-wide)
    PE:  X'[hd] += V'[jb].T @ pt-slices (PSUM accumulation)
  V' has a ones-column per head block so X' row 64 = denom - N.
  X' is init'd with Vsum via matmul of Wv' against broadcast hsum.
- Finalize per head uses 1/(N+d) ~= 1/N - d/N^2 (|d|<~20, rel err <2e-5):
  one 1-lane tensor_scalar + gpsimd partition_broadcast + one tensor_tensor.
- FFN: relu done on DVE (tensor_scalar add-bias + max0); row softmax skips
  max-subtraction (|logits| < 0.01).
"""
import sys
import numpy as np

sys.path.insert(0, "/opt/trn_rl_repo")
import ml_dtypes  # noqa: E402

IN = 256
H = 4
DH = 64
NCORES = 8
F1 = 512
DOUT = 256
SCALE = 1.0 / 16.0  # 1/sqrt(IN)
BF16 = ml_dtypes.bfloat16

_cache = {}


def build(n_nodes=4096, qs=512):
    key = (n_nodes, qs)
    if key in _cache:
        return _cache[key]

    from contextlib import ExitStack
    import concourse.tile as tile
    from concourse import mybir, bacc
    from concourse.alu_op_type import AluOpType

    fp32, bf16 = mybir.dt.float32, mybir.dt.bfloat16
    AF = mybir.ActivationFunctionType
    AX = mybir.AxisListType

    NJB = n_nodes // 128          # 128-row j blocks (32)
    NJC = NJB // 8                # 1MB adj chunks per head (4)
    NQC = qs // 128               # 128-row q chunks (4)
    VW = 65                       # v block width per head (64 + ones col)
    RN = 1.0 / float(n_nodes)     # 1/4096
    RN2 = RN * RN

    nc = bacc.Bacc("TRN2", target_bir_lowering=False, debug=False,
                   enable_asserts=False)

    adj_d = nc.dram_tensor("adjc", [128, H * NJB * qs], bf16,
                           kind="ExternalInput").ap()
    hT_d = nc.dram_tensor("hT", [IN, n_nodes], bf16, kind="ExternalInput").ap()
    hTq_d = nc.dram_tensor("hTq", [IN, qs], bf16, kind="ExternalInput").ap()
    hs_d = nc.dram_tensor("hs", [IN, 1], bf16, kind="ExternalInput").ap()
    wq_d = nc.dram_tensor("wq", [IN, H * DH], bf16, kind="ExternalInput").ap()
    wk_d = nc.dram_tensor("wk", [IN, H * DH], bf16, kind="ExternalInput").ap()
    wv_d = nc.dram_tensor("wv", [IN, H * VW], bf16, kind="ExternalInput").ap()
    w1_d = nc.dram_tensor("w1", [IN, F1], bf16, kind="ExternalInput").ap()
    w2_d = nc.dram_tensor("w2", [F1, DOUT], bf16, kind="ExternalInput").ap()
    b1_d = nc.dram_tensor("b1", [128, F1 // 128], fp32, kind="ExternalInput").ap()
    b2_d = nc.dram_tensor("b2", [1, DOUT], fp32, kind="ExternalInput").ap()
    out_d = nc.dram_tensor("out", [qs, DOUT], fp32, kind="ExternalOutput").ap()

    with ExitStack() as ctx:
        tc = ctx.enter_context(tile.TileContext(nc))
        pc = ctx.enter_context(tc.tile_pool(name="const", bufs=1))
        pst = ctx.enter_context(tc.tile_pool(name="stp", bufs=3, space="PSUM"))
        pxt = ctx.enter_context(tc.tile_pool(name="xtp", bufs=2, space="PSUM"))
        pa = ctx.enter_context(tc.tile_pool(name="adjp", bufs=4))
        pe_ = ctx.enter_context(tc.tile_pool(name="ep", bufs=2))
        pm1 = ctx.enter_context(tc.tile_pool(name="m1p", bufs=2))
        ppt = ctx.enter_context(tc.tile_pool(name="ptp", bufs=2))
        psm = ctx.enter_context(tc.tile_pool(name="smallp", bufs=2))

        # ---------------- input DMAs ----------------
        # order on the sync HWDGE ring = completion order: small critical
        # prep inputs first (qT needs hTq+wq), then hT (kT/v), then the rest;
        # adj chunks queue behind all of these.
        hTq_sb = [pc.tile([128, qs], bf16, tag=f"hTq{dc}", name=f"hTq{dc}")
                  for dc in range(2)]
        for dc in range(2):
            nc.sync.dma_start(out=hTq_sb[dc][:], in_=hTq_d[dc * 128:(dc + 1) * 128, :])
        wq_sb = pc.tile([128, 2 * H * DH], bf16, tag="wq")
        wk_sb = pc.tile([128, 2 * H * DH], bf16, tag="wk")
        for sb, d in ((wq_sb, wq_d), (wk_sb, wk_d)):
            for dc in range(2):
                nc.sync.dma_start(out=sb[:, dc * 256:(dc + 1) * 256],
                                    in_=d[dc * 128:(dc + 1) * 128, :])
        hT_sb = [pc.tile([128, n_nodes], bf16, tag=f"hT{dc}", name=f"hT{dc}")
                 for dc in range(2)]
        for dc in range(2):
            nc.sync.dma_start(out=hT_sb[dc][:], in_=hT_d[dc * 128:(dc + 1) * 128, :])
        wv_sb = pc.tile([128, 2 * H * VW], bf16, tag="wv")
        for dc in range(2):
            nc.sync.dma_start(out=wv_sb[:, dc * 260:(dc + 1) * 260],
                                in_=wv_d[dc * 128:(dc + 1) * 128, :])
        w1_sb = [pc.tile([128, F1], bf16, tag=f"w1_{dc}", name=f"w1_{dc}")
                 for dc in range(2)]
        for dc in range(2):
            nc.sync.dma_start(out=w1_sb[dc][:], in_=w1_d[dc * 128:(dc + 1) * 128, :])
        w2_sb = pc.tile([128, 4 * DOUT], bf16, tag="w2")
        for fc in range(4):
            nc.sync.dma_start(out=w2_sb[:, fc * DOUT:(fc + 1) * DOUT],
                                in_=w2_d[fc * 128:(fc + 1) * 128, :])
        b1_sb = pc.tile([128, F1 // 128], fp32, tag="b1")
        nc.sync.dma_start(out=b1_sb[:], in_=b1_d[:, :])
        b2_sb = pc.tile([1, DOUT], fp32, tag="b2")
        nc.sync.dma_start(out=b2_sb[:], in_=b2_d[:, :])
        b2_bc = pc.tile([128, DOUT], fp32, tag="b2_bc")
        nc.gpsimd.partition_broadcast(b2_bc[:], b2_sb[0:1, :])

        # ---------------- projections ----------------
        # q^T / k^T: head pairs packed on partitions (pair p -> heads 2p,2p+1)
        qT_sb = [pc.tile([128, qs], bf16, tag=f"qT{p}", name=f"qT{p}") for p in range(2)]
        qps = pst.tile([128, 1024], fp32, tag="st")
        for p in range(2):
            for dc in range(2):
                nc.tensor.matmul(qps[:, p * 512:(p + 1) * 512],
                                 wq_sb[:, dc * 256 + p * 128: dc * 256 + (p + 1) * 128],
                                 hTq_sb[dc][:],
                                 start=(dc == 0), stop=(dc == 1))
        for p in range(2):
            nc.scalar.activation(qT_sb[p][:], qps[:, p * 512:(p + 1) * 512], AF.Copy)
        # kT: wide rhs (N=1024), copies on ACT (idle during prep)
        kT_sb = [pc.tile([128, n_nodes], bf16, tag=f"kT{p}", name=f"kT{p}") for p in range(2)]
        for p in range(2):
            for jt in range(n_nodes // 1024):
                ps = pst.tile([128, 1024], fp32, tag="st")
                for s in range(2):
                    for dc in range(2):
                        nc.tensor.matmul(ps[:, s * 512:(s + 1) * 512],
                                         wk_sb[:, dc * 256 + p * 128: dc * 256 + (p + 1) * 128],
                                         hT_sb[dc][:, jt * 1024 + s * 512: jt * 1024 + (s + 1) * 512],
                                         start=(dc == 0), stop=(dc == 1))
                nc.scalar.activation(kT_sb[p][:, jt * 1024:(jt + 1) * 1024], ps[:], AF.Copy)
        # v natural [128j, NJB*260] bf16; block jb cols jb*260 + hd*65 + f,
        # col jb*260 + hd*65 + 64 = 1.0 (ones col; wv has zeros there)
        v_sb = pc.tile([128, NJB * H * VW], bf16, tag="v")
        for jb2 in range(NJB // 2):
            ps = pst.tile([128, 520], fp32, tag="st")
            for s in range(2):
                jb = jb2 * 2 + s
                for dc in range(2):
                    nc.tensor.matmul(ps[:, s * 260:(s + 1) * 260],
                                     hT_sb[dc][:, jb * 128:(jb + 1) * 128],
                                     wv_sb[:, dc * 260:(dc + 1) * 260],
                                     start=(dc == 0), stop=(dc == 1))
            nc.vector.tensor_copy(v_sb[:, jb2 * 520:(jb2 + 1) * 520], ps[:])
        # set every ones-col (all cols == 64 mod 65) to 1.0 in one strided memset
        nc.gpsimd.memset(v_sb[:, 64::65], 1.0)

        # ---------------- Vsum = Wv'.T @ hsum (hsum shipped from host) ----
        hsum_bf = [psm.tile([128, 1], bf16, tag=f"hsumb{dc}", name=f"hsumb{dc}")
                   for dc in range(2)]
        for dc in range(2):
            nc.sync.dma_start(out=hsum_bf[dc][:], in_=hs_d[dc * 128:(dc + 1) * 128, :])
        # Vsum[hd] = sum_j v'[j, hd, :] as [65, 1] per-partition columns
        vs_ps = pst.tile([VW, H], fp32, tag="st")
        for hd in range(H):
            for dc in range(2):
                nc.tensor.matmul(vs_ps[:, hd:hd + 1],
                                 wv_sb[:, dc * 260 + hd * VW: dc * 260 + hd * VW + VW],
                                 hsum_bf[dc][:],
                                 start=(dc == 0), stop=(dc == 1))
        vs_sb = psm.tile([VW, H], fp32, tag="vs_sb")
        nc.vector.tensor_copy(vs_sb[:], vs_ps[:])


        # ---------------- attention ----------------
        embT = [pc.tile([128, qs], bf16, tag=f"embT{p}", name=f"embT{p}") for p in range(2)]

        for pr in range(2):
            xts = [pxt.tile([VW, qs], fp32, tag="xt", name=f"xt{2 * pr + i}")
                   for i in range(2)]
            for jc in range(NJC):
                acs = []
                for i in range(2):
                    ac = pa.tile([128, 8 * 512], bf16, tag="adj")
                    base = ((2 * pr + i) * NJC + jc) * 8 * 512
                    nc.sync.dma_start(out=ac[:], in_=adj_d[:, base: base + 8 * 512])
                    acs.append(ac)
                es = [pe_.tile([128, 4096], bf16, tag="e",
                               name=f"e{pr}_{jc}_{i}") for i in range(2)]
                # 16 row-tiled S^T matmuls (heads 2pr/2pr+1 run on row
                # tiles (0,0)/(64,0) of the PE concurrently)
                for sub in range(4):
                    for i in range(2):
                        off = i * 64
                        st = pst.tile([128, 1024], fp32, tag="st")
                        for k in range(2):
                            jb = jc * 8 + sub * 2 + k
                            nc.tensor.matmul(st[:, k * 512:(k + 1) * 512],
                                             kT_sb[pr][off:off + 64, jb * 128:(jb + 1) * 128],
                                             qT_sb[pr][off:off + 64, :],
                                             start=True, stop=True,
                                             tile_position=(off, 0))
                        nc.scalar.activation(es[i][:, sub * 1024:(sub + 1) * 1024],
                                             st[:], AF.Exp, scale=SCALE)
                for i in range(2):
                    hd = 2 * pr + i
                    em1 = pm1.tile([128, 4096], bf16, tag="em1")
                    pt = ppt.tile([128, 4096], bf16, tag="pt")
                    for h2 in range(2):
                        lo, hi = h2 * 2048, (h2 + 1) * 2048
                        nc.vector.tensor_scalar_sub(em1[:, lo:hi], es[i][:, lo:hi], 1.0)
                        nc.vector.tensor_tensor(pt[:, lo:hi], em1[:, lo:hi],
                                                acs[i][:, lo:hi], AluOpType.mult)
                        for k in range(4):
                            jb = jc * 8 + h2 * 4 + k
                            kk = h2 * 4 + k
                            nc.tensor.matmul(xts[i][:],
                                             v_sb[:, jb * 260 + hd * VW: jb * 260 + hd * VW + VW],
                                             pt[:, kk * 512:(kk + 1) * 512],
                                             start=(jb == 0), stop=(jb == NJB - 1))
            for i in range(2):
                hd = 2 * pr + i
                off = i * 64
                # finalize: emb^T rows = (X'' + Vsum) * (1/N - d/N^2), d = X''[64]
                rec1 = psm.tile([1, qs], fp32, tag="rec1")
                nc.vector.tensor_scalar(rec1[:], xts[i][64:65, :], -RN2, RN,
                                        op0=AluOpType.mult, op1=AluOpType.add)
                recb = psm.tile([64, qs], fp32, tag="recb")
                nc.gpsimd.partition_broadcast(recb[:], rec1[0:1, :])
                nc.vector.scalar_tensor_tensor(embT[pr][off:off + 64, :], xts[i][0:64, :],
                                               vs_sb[0:64, hd:hd + 1], recb[:],
                                               AluOpType.add, AluOpType.mult)

        # ---------------- FFN + row softmax ----------------
        p1_sb = pc.tile([128, (F1 // 128) * qs], bf16, tag="p1")
        for fc2 in range(2):
            ps = pst.tile([128, 1024], fp32, tag="st")
            for s in range(2):
                fc = fc2 * 2 + s
                for dc in range(2):
                    nc.tensor.matmul(ps[:, s * 512:(s + 1) * 512],
                                     w1_sb[dc][:, fc * 128:(fc + 1) * 128],
                                     embT[dc][:], start=(dc == 0), stop=(dc == 1))
            for s in range(2):
                fc = fc2 * 2 + s
                nc.vector.tensor_scalar(p1_sb[:, fc * qs:(fc + 1) * qs],
                                        ps[:, s * 512:(s + 1) * 512],
                                        b1_sb[:, fc:fc + 1], 0.0,
                                        op0=AluOpType.add, op1=AluOpType.max)
        for qc in range(NQC):
            ps2 = pst.tile([128, DOUT], fp32, tag="st")
            for fc in range(F1 // 128):
                nc.tensor.matmul(ps2[:],
                                 p1_sb[:, fc * qs + qc * 128: fc * qs + (qc + 1) * 128],
                                 w2_sb[:, fc * DOUT:(fc + 1) * DOUT],
                                 start=(fc == 0), stop=(fc == F1 // 128 - 1))
            t2 = psm.tile([128, DOUT], fp32, tag="t2")
            nc.vector.tensor_tensor(t2[:], ps2[:], b2_bc[:], AluOpType.add)
            e2 = psm.tile([128, DOUT], fp32, tag="e2")
            sm = psm.tile([128, 1], fp32, tag="sm")
            nc.scalar.activation(e2[:], t2[:], AF.Exp, accum_out=sm[:])
            rc2 = psm.tile([128, 1], fp32, tag="rc2")
            nc.vector.reciprocal(rc2[:], sm[:])
            o = psm.tile([128, DOUT], fp32, tag="o")
            nc.vector.tensor_scalar_mul(o[:], e2[:], rc2[:])
            nc.sync.dma_start(out=out_d[qc * 128:(qc + 1) * 128, :], in_=o[:])

    nc.compile()
    _cache[key] = nc
    return nc


def make_in_maps(h, adj, Wq, Wk, Wv, W1, b1, W2, b2, n_nodes, qs, ncores):
    h = np.asarray(h, np.float32)
    adj = np.asarray(adj, np.float32)
    hT = np.ascontiguousarray(h.T.astype(BF16))
    hs = h.sum(0, dtype=np.float64).astype(np.float32).astype(BF16).reshape(IN, 1)
    WqP = np.ascontiguousarray(
        np.asarray(Wq, np.float32).transpose(1, 0, 2).reshape(IN, H * DH)).astype(BF16)
    WkP = np.ascontiguousarray(
        np.asarray(Wk, np.float32).transpose(1, 0, 2).reshape(IN, H * DH)).astype(BF16)
    WvT = np.asarray(Wv, np.float32).transpose(1, 0, 2)  # [IN, H, DH]
    WvP = np.zeros((IN, H * 65), dtype=BF16)
    for hd in range(H):
        WvP[:, hd * 65: hd * 65 + 64] = WvT[:, hd, :].astype(BF16)
    W1b = np.asarray(W1, np.float32).astype(BF16)
    W2b = np.asarray(W2, np.float32).astype(BF16)
    b1r = np.ascontiguousarray(np.asarray(b1, np.float32).reshape(F1 // 128, 128).T)
    b2r = np.asarray(b2, np.float32).reshape(1, DOUT)
    # adj -> per-core SBUF-ready layout [128, hd*NJB*qs + jb*qs + q] (bf16)
    au = adj.astype(BF16).view(np.uint16)  # [H, N, N]
    NJB = n_nodes // 128
    in_maps = []
    for c in range(ncores):
        q0 = c * qs
        A = au[:, q0:q0 + qs, :]                       # [H, qs, N] view
        R = A.reshape(H, qs, NJB, 128).transpose(3, 0, 2, 1)  # [128, H, NJB, qs]
        adjc = np.ascontiguousarray(R).reshape(128, H * NJB * qs).view(BF16)
        in_maps.append({
            "adjc": adjc,
            "hT": hT,
            "hTq": np.ascontiguousarray(hT[:, q0:q0 + qs]),
            "hs": hs,
            "wq": WqP, "wk": WkP, "wv": WvP,
            "w1": W1b, "w2": W2b, "b1": b1r, "b2": b2r,
        })
    return in_maps


def kernel(h, adj, Wq, Wk, Wv, W1, b1, W2, b2):
    import os
    n_nodes, qs = 4096, 512
    nc = build(n_nodes, qs)
    from concourse.bass_utils import run_bass_kernel_spmd
    in_maps = make_in_maps(h, adj, Wq, Wk, Wv, W1, b1, W2, b2, n_nodes, qs, NCORES)
    trace = bool(os.environ.get("BASS_KERNEL_TRACE"))
    res = run_bass_kernel_spmd(nc, in_maps, list(range(NCORES)), trace=trace)
    if trace and res.exec_time_ns is not None:
        print(f"HW exec time: {res.exec_time_ns} ns")
        kernel.last_exec_time_ns = res.exec_time_ns
    out = np.concatenate([np.asarray(res.results[c]["out"]) for c in range(NCORES)],
                         axis=0)
    return out.astype(np.float32)
